# revision 1
# baseline (speedup 1.0000x reference)
"""ClusterAttn Trainium2 kernel (Bass/Tile), 8-way data parallel over batch.

Full inputs in, full outputs out. Internally:
  - batch B=32 is split 4-per-core across 8 NeuronCores (pure DP).
  - all PE compute in bf16 (fp32 psum accumulate); host casts inputs.
  - act/gate GEMM folded: Wcomb = W_exp @ [BN1-folded cluster_weights | W_ga]
    contracts over D=768 straight from the transposed-x tiles, so fea never
    needs a PE transpose and the cluster GEMM halves.
  - x is pre-transposed on the host to [b, ko, ki, s] so each batch loads
    with ONE fully-contiguous-per-partition DMA; weights likewise arrive in
    their SBUF layout (one descriptor row per partition). Output is staged
    per batch in SBUF and stored with one DMA per batch.
  - attention re-associated: scores = x @ (Wq @ k^T), out = attn @ (v @ Wp2).
"""

from contextlib import ExitStack

import numpy as np
import ml_dtypes

import concourse.bass as bass
import concourse.bacc as bacc
import concourse.tile as tile
import concourse.mybir as mybir
from concourse import bass_utils
from concourse.masks import make_identity

dt = mybir.dt
AF = mybir.ActivationFunctionType
ALU = mybir.AluOpType

EPS = 1e-5
N_CORES = 8
B, S, D = 32, 1024, 768
E, G, C, P = 2, 8, 64, 384
EF = E * D            # 1536
GC = G * C            # 512
GCG = GC + G          # 520
GFS = EF // G         # 192
NB = B // N_CORES     # batches per core
NT = S // 128         # token tiles per batch
KD = D // 128         # 6 contraction k-tiles over D
F32 = dt.float32
BF16 = dt.bfloat16
BF16NP = ml_dtypes.bfloat16


def build_program(flags):
    has_bexp, has_bq, has_bkv, has_bp2 = flags
    nc = bacc.Bacc(
        "TRN2",
        debug=False,
        enable_asserts=False,
        num_devices=N_CORES,
    )

    # x pre-transposed on host: xt[b, ko, ki, s] = x[b, s, ko*128+ki]
    xt_d = nc.dram_tensor("xt", (NB, KD, 128, S), BF16, kind="ExternalInput").ap()
    out_d = nc.dram_tensor("out", (NB, S, D), F32, kind="ExternalOutput").ap()
    # weights already in SBUF layout [ki, ko, n]
    wexp_d = nc.dram_tensor("wexp", (128, KD, EF), BF16, kind="ExternalInput").ap()
    wcomb_d = nc.dram_tensor("wcomb", (128, KD, GCG), BF16, kind="ExternalInput").ap()
    bias1_d = nc.dram_tensor("bias1", (GCG,), F32, kind="ExternalInput").ap()
    wproj_d = nc.dram_tensor("wproj", (128, 2, D), BF16, kind="ExternalInput").ap()
    s2_d = nc.dram_tensor("s2", (C, 1), F32, kind="ExternalInput").ap()
    bias2_d = nc.dram_tensor("bias2", (C, D), F32, kind="ExternalInput").ap()
    wkv_d = nc.dram_tensor("wkv", (128, KD, 2 * P), BF16, kind="ExternalInput").ap()
    wqT_d = nc.dram_tensor("wqT", (128, 3, D), BF16, kind="ExternalInput").ap()
    wp2_d = nc.dram_tensor("wp2", (128, 3, D), BF16, kind="ExternalInput").ap()
    bexp_d = bq_d = bkv_d = bp2_d = None
    if has_bexp:
        bexp_d = nc.dram_tensor("bexp", (EF,), F32, kind="ExternalInput").ap()
    if has_bq:
        bq_d = nc.dram_tensor("bqT", (128, 3, 1), BF16, kind="ExternalInput").ap()
    if has_bkv:
        bkv_d = nc.dram_tensor("bkv", (2 * P,), F32, kind="ExternalInput").ap()
    if has_bp2:
        bp2_d = nc.dram_tensor("bp2", (D,), F32, kind="ExternalInput").ap()

    with tile.TileContext(nc) as tc, ExitStack() as ctx:
        # ---------------- pools ----------------
        const = ctx.enter_context(tc.tile_pool(name="const", bufs=1))
        mid = ctx.enter_context(tc.tile_pool(name="mid", bufs=1))
        p_xt = ctx.enter_context(tc.tile_pool(name="p_xt", bufs=3))
        p_fea = ctx.enter_context(tc.tile_pool(name="p_fea", bufs=3))
        p_act = ctx.enter_context(tc.tile_pool(name="p_act", bufs=2))
        p_sm = ctx.enter_context(tc.tile_pool(name="p_sm", bufs=4))
        p_out = ctx.enter_context(tc.tile_pool(name="p_out", bufs=2))
        p_cacc = ctx.enter_context(tc.tile_pool(name="p_cacc", bufs=2))
        ps_mm = ctx.enter_context(tc.tile_pool(name="ps_mm", bufs=4, space="PSUM"))
        ps_t = ctx.enter_context(tc.tile_pool(name="ps_t", bufs=2, space="PSUM"))
        ps_c = ctx.enter_context(tc.tile_pool(name="ps_c", bufs=2, space="PSUM"))

        # ---------------- x prefetch + constants ----------------
        x_store = {}
        cent_store = {}

        def load_xt_batch(b):
            xt = p_xt.tile([128, KD, S], BF16, tag="xt", name=f"xt{b}")
            nc.sync.dma_start(xt[:], xt_d[b].rearrange("ko ki s -> ki ko s"))
            x_store[b] = xt
            cent_store[b] = ps_c.tile([128, 384], F32, tag="cent", name=f"centps{b}")
            return xt

        load_xt_batch(0)

        ident = const.tile([128, 128], BF16)
        ident_f = const.tile([128, 128], F32)
        make_identity(nc, ident_f[:])
        nc.vector.tensor_copy(ident[:], ident_f[:])

        wexp_sb = const.tile([128, KD, EF], BF16)
        nc.sync.dma_start(wexp_sb[:], wexp_d)
        wcomb_sb = const.tile([128, KD, GCG], BF16)
        nc.sync.dma_start(wcomb_sb[:], wcomb_d)
        bias1_sb = const.tile([128, GCG], F32)
        nc.gpsimd.dma_start(bias1_sb[:], bias1_d.partition_broadcast(128))
        wproj_sb = const.tile([128, 2, D], BF16)
        nc.sync.dma_start(wproj_sb[:], wproj_d)
        s2_sb = const.tile([C, 1], F32)
        nc.sync.dma_start(s2_sb[:], s2_d)
        bias2_sb = const.tile([C, D], F32)
        nc.sync.dma_start(bias2_sb[:], bias2_d)
        wkv_sb = const.tile([128, KD, 2 * P], BF16)
        nc.sync.dma_start(wkv_sb[:], wkv_d)
        wqT_sb = const.tile([128, 3, D], BF16)
        nc.sync.dma_start(wqT_sb[:], wqT_d)
        wp2_sb = const.tile([128, 3, D], BF16)
        nc.sync.dma_start(wp2_sb[:], wp2_d)
        if has_bexp:
            bexp_sb = const.tile([128, EF], F32)
            nc.gpsimd.dma_start(bexp_sb[:], bexp_d.partition_broadcast(128))
        if has_bq:
            bq_sb = const.tile([128, 3, 1], BF16)
            nc.sync.dma_start(bq_sb[:], bq_d)
            ones_sb = const.tile([1, 128], BF16)
            nc.vector.memset(ones_sb[:], 1.0)
        if has_bkv:
            bkv_sb = const.tile([C, 2 * P], F32)
            nc.gpsimd.dma_start(bkv_sb[:], bkv_d.partition_broadcast(C))
        if has_bp2:
            bp2_sb = const.tile([128, D], F32)
            nc.gpsimd.dma_start(bp2_sb[:], bp2_d.partition_broadcast(128))

        def transpose_to(out_ps, in_ap, start=True, stop=True):
            """PE transpose of bf16 in_ap -> fp32 psum tile slice."""
            kp = in_ap.partition_size()
            nc.tensor.matmul(
                out_ps,
                in_ap,
                ident[0:kp, 0:kp],
                is_transpose=True,
                start=start,
                stop=stop,
                skip_group_check=True,
            )

        seg = {}
        inv_sqrt_p = float(1.0 / np.sqrt(np.float32(P)))

        def seg_F(b, t):
            """fea GEMM + act/gate GEMM, all from xt; fea copies to SBUF."""
            xt = x_store[b]
            xk = lambda k: xt[:, k, t * 128:(t + 1) * 128]
            fea = p_fea.tile([128, EF], BF16, tag="fea")
            for n3 in range(3):
                fp = ps_mm.tile([128, 512], F32, tag="mm")
                for k in range(KD):
                    nc.tensor.matmul(
                        fp[:], xk(k),
                        wexp_sb[:, k, n3 * 512:(n3 + 1) * 512],
                        start=(k == 0), stop=(k == KD - 1))
                dst = fea[:, n3 * 512:(n3 + 1) * 512]
                if has_bexp:
                    nc.vector.tensor_add(dst, fp[:],
                                         bexp_sb[:, n3 * 512:(n3 + 1) * 512])
                else:
                    nc.scalar.copy(dst, fp[:])
            seg[(b, t)] = {"fea": fea}
            aps = []
            for a0, an in ((0, 256), (256, 264)):
                apm = ps_mm.tile([128, 264], F32, tag="mm")
                for k in range(KD):
                    nc.tensor.matmul(
                        apm[:, 0:an], xk(k),
                        wcomb_sb[:, k, a0:a0 + an],
                        start=(k == 0), stop=(k == KD - 1))
                aps.append((apm, a0, an))
            seg[(b, t)]["aps"] = aps

        def seg_S(b, t):
            """grouped softmax * sigmoid gate -> actf (bf16)."""
            st = seg[(b, t)]
            act = p_act.tile([128, GCG], F32, tag="act")
            for apm, a0, an in st.pop("aps"):
                nc.vector.tensor_add(act[:, a0:a0 + an], apm[:, 0:an],
                                     bias1_sb[:, a0:a0 + an])
            e = p_act.tile([128, GC], F32, tag="e")
            nc.scalar.activation(e[:], act[:, 0:GC], AF.Exp)
            ssum = p_sm.tile([128, G], F32, tag="ssum")
            nc.vector.reduce_sum(ssum[:], e[:].rearrange("p (g c) -> p g c", g=G),
                                 axis=mybir.AxisListType.X)
            eneg = p_sm.tile([128, G], F32, tag="eneg")
            nc.scalar.activation(eneg[:], act[:, GC:GCG], AF.Exp, scale=-1.0)
            nc.vector.tensor_scalar_add(eneg[:], eneg[:], 1.0)
            ga = p_sm.tile([128, G], F32, tag="ga")
            nc.vector.reciprocal(ga[:], eneg[:])
            rs = p_sm.tile([128, G], F32, tag="rs")
            nc.vector.reciprocal(rs[:], ssum[:])
            nc.vector.tensor_mul(rs[:], rs[:], ga[:])
            actf = p_act.tile([128, GC], BF16, tag="actf")
            nc.vector.tensor_tensor(
                out=actf[:].rearrange("p (g c) -> p g c", g=G),
                in0=e[:].rearrange("p (g c) -> p g c", g=G),
                in1=rs[:].unsqueeze(2).broadcast_to((128, G, C)),
                op=ALU.mult)
            st["actf"] = actf

        def seg_C(b, t):
            st = seg.pop((b, t))
            fea, actf = st["fea"], st["actf"]
            cp = cent_store[b]
            # all 4 diagonal blocks accumulate into ONE psum bank; the two
            # extracted sub-blocks never overlap, so psum accumulation
            # performs the over-block reduction for free.
            for mi in range(4):
                nc.tensor.matmul(
                    cp[:], actf[:, mi * 128:(mi + 1) * 128],
                    fea[:, mi * 384:(mi + 1) * 384],
                    start=(t == 0 and mi == 0),
                    stop=(t == NT - 1 and mi == 3),
                    skip_group_check=True)

        def make_items(b):
            """Pipelined mid-phase chunks + pass2 a/b halves for batch b.
            Each chunk's PE work depends only on DVE/ACT output emitted at
            least one pass1-tile earlier, so the PE FIFO never blocks."""
            mt = {}

            def mid_cent():
                cp = cent_store.pop(b)
                cent = p_cacc.tile([C, GFS], BF16, tag="cacc")
                tmp = p_cacc.tile([C, GFS], F32, tag="ctmp")
                nc.scalar.copy(tmp[:], cp[0:64, 0:192])
                nc.vector.tensor_add(cent[:], tmp[:], cp[64:128, 192:384])
                mt["cent"] = cent

            def mid1():
                cent = mt["cent"]
                centT = mid.tile([128, 2, C], BF16, tag="centT")
                ctp = ps_t.tile([128, 512], BF16, tag="t")
                transpose_to(ctp[:, 0:64], cent[:, 0:128])
                transpose_to(ctp[0:64, 64:128], cent[:, 128:192])
                nc.vector.tensor_copy(centT[:, 0, :], ctp[:, 0:64])
                nc.vector.tensor_copy(centT[0:64, 1, :], ctp[0:64, 64:128])
                # nc2 = BN2(cent @ W_proj + b_proj)  [64, 768]
                nc2 = mid.tile([C, D], BF16, tag="nc2")
                for n0, nn in ((0, 512), (512, 256)):
                    np_ps = ps_mm.tile([128, 512], F32, tag="mm")
                    nc.tensor.matmul(np_ps[0:C, 0:nn], centT[:, 0, :],
                                     wproj_sb[:, 0, n0:n0 + nn], start=True, stop=False)
                    nc.tensor.matmul(np_ps[0:C, 0:nn], centT[0:64, 1, :],
                                     wproj_sb[0:64, 1, n0:n0 + nn], start=False, stop=True)
                    nc.vector.scalar_tensor_tensor(
                        out=nc2[:, n0:n0 + nn], in0=np_ps[0:C, 0:nn], scalar=s2_sb[:, 0:1],
                        in1=bias2_sb[:, n0:n0 + nn], op0=ALU.mult, op1=ALU.add)
                mt["nc2"] = nc2

            def mid2():
                nc2 = mt["nc2"]
                nc2T = mid.tile([128, KD, C], BF16, tag="nc2T")
                for grp in range(2):
                    ntp = ps_t.tile([128, 512], BF16, tag="t")
                    for i in range(3):
                        transpose_to(ntp[:, i * 64:(i + 1) * 64],
                                     nc2[:, (grp * 3 + i) * 128:(grp * 3 + i + 1) * 128],
                                     start=(i == 0), stop=(i == 2))
                    nc.vector.tensor_copy(
                        nc2T[:, grp * 3:(grp + 1) * 3, :].rearrange("p a b -> p (a b)"),
                        ntp[:, 0:192])
                kv = mid.tile([C, 2 * P], BF16, tag="kv")
                for n0, nn in ((0, 512), (512, 256)):
                    kv_ps = ps_mm.tile([128, 512], F32, tag="mm")
                    for k in range(KD):
                        nc.tensor.matmul(kv_ps[0:C, 0:nn], nc2T[:, k, :],
                                         wkv_sb[:, k, n0:n0 + nn],
                                         start=(k == 0), stop=(k == KD - 1))
                    if has_bkv:
                        nc.vector.tensor_add(kv[:, n0:n0 + nn], kv_ps[0:C, 0:nn],
                                             bkv_sb[:, n0:n0 + nn])
                    else:
                        nc.scalar.copy(kv[:, n0:n0 + nn], kv_ps[0:C, 0:nn])
                mt["kv"] = kv

            def mid3():
                kv = mt["kv"]
                kT = mid.tile([128, 3, C], BF16, tag="kT")
                vT = mid.tile([128, 3, C], BF16, tag="vT")
                ktp = ps_t.tile([128, 512], BF16, tag="t")
                for i in range(3):
                    transpose_to(ktp[:, i * 64:(i + 1) * 64], kv[:, i * 128:(i + 1) * 128],
                                 start=(i == 0), stop=(i == 2))
                nc.vector.tensor_copy(kT[:].rearrange("p a b -> p (a b)"), ktp[:, 0:192])
                vtp = ps_t.tile([128, 512], BF16, tag="t")
                for i in range(3):
                    transpose_to(vtp[:, i * 64:(i + 1) * 64],
                                 kv[:, P + i * 128:P + (i + 1) * 128],
                                 start=(i == 0), stop=(i == 2))
                nc.vector.tensor_copy(vT[:].rearrange("p a b -> p (a b)"), vtp[:, 0:192])
                mt["kT"], mt["vT"] = kT, vT

            def mid4():
                kT, vT = mt["kT"], mt["vT"]
                wqk = mid.tile([128, KD, C], BF16, tag="wqk")
                for m in range(KD):
                    wq_ps = ps_t.tile([128, 512], F32, tag="t")
                    for k3 in range(3):
                        nc.tensor.matmul(wq_ps[:, 0:C], wqT_sb[:, k3, m * 128:(m + 1) * 128],
                                         kT[:, k3, :], start=(k3 == 0), stop=(k3 == 2))
                    nc.scalar.copy(wqk[:, m, :], wq_ps[:, 0:C])
                mt["bias_c"] = None
                if has_bq:
                    bc_ps = ps_t.tile([128, 512], F32, tag="t")
                    for k3 in range(3):
                        nc.tensor.matmul(bc_ps[0:1, 0:C], bq_sb[:, k3, :],
                                         kT[:, k3, :], start=(k3 == 0), stop=(k3 == 2))
                    bias_c = mid.tile([1, C], BF16, tag="bias_c")
                    nc.scalar.copy(bias_c[:], bc_ps[0:1, 0:C])
                    mt["bias_c"] = bias_c
                vw = mid.tile([C, D], BF16, tag="vw")
                for n0, nn in ((0, 512), (512, 256)):
                    vw_ps = ps_mm.tile([128, 512], F32, tag="mm")
                    for k3 in range(3):
                        nc.tensor.matmul(vw_ps[0:C, 0:nn], vT[:, k3, :],
                                         wp2_sb[:, k3, n0:n0 + nn],
                                         start=(k3 == 0), stop=(k3 == 2))
                    nc.scalar.copy(vw[:, n0:n0 + nn], vw_ps[0:C, 0:nn])
                mt["wqk"], mt["vw"] = wqk, vw
                mt["out_sb"] = p_out.tile([128, NT, D], F32, tag="out", name=f"out{b}")

            def p2a(t):
                """scores + softmax -> attn(t); PE part is tiny (N=64 MMs)."""
                wqk, bias_c = mt["wqk"], mt["bias_c"]
                xt = x_store[b]
                sc_ps = ps_t.tile([128, 512], F32, tag="t")
                for k in range(KD):
                    nc.tensor.matmul(sc_ps[:, 0:C],
                                     xt[:, k, t * 128:(t + 1) * 128], wqk[:, k, :],
                                     start=(k == 0), stop=(k == KD - 1 and not has_bq),
                                     skip_group_check=True)
                if has_bq:
                    nc.tensor.matmul(sc_ps[:, 0:C], ones_sb[:], bias_c[:],
                                     start=False, stop=True, skip_group_check=True)
                e_att = p_sm.tile([128, C], F32, tag="e_att")
                ssum_a = p_sm.tile([128, 1], F32, tag="ssum_a")
                nc.scalar.activation(e_att[:], sc_ps[:, 0:C], AF.Exp,
                                     scale=inv_sqrt_p, accum_out=ssum_a[:])
                rs_a = p_sm.tile([128, 1], F32, tag="rs_a")
                nc.vector.reciprocal(rs_a[:], ssum_a[:])
                attn = p_sm.tile([128, C], BF16, tag="attn")
                nc.vector.tensor_scalar_mul(attn[:], e_att[:], rs_a[:])
                mt[("attn", t)] = attn

            def p2b(t):
                """attn transpose + out GEMM + out staging; runs >=1 slot
                after p2a(t) so attn is ready when the PE reaches it."""
                vw, out_sb = mt["vw"], mt["out_sb"]
                attn = mt.pop(("attn", t))
                at_ps = ps_t.tile([128, 512], BF16, tag="t")
                transpose_to(at_ps[0:C, 0:128], attn[:])
                attnT = p_sm.tile([C, 128], BF16, tag="attnT")
                nc.vector.tensor_copy(attnT[:], at_ps[0:C, 0:128])
                for n0, nn in ((0, 512), (512, 256)):
                    fo_ps = ps_mm.tile([128, 512], F32, tag="mm")
                    nc.tensor.matmul(fo_ps[:, 0:nn], attnT[:],
                                     vw[:, n0:n0 + nn], start=True, stop=True)
                    if has_bp2:
                        nc.vector.tensor_add(out_sb[:, t, n0:n0 + nn], fo_ps[:, 0:nn],
                                             bp2_sb[:, n0:n0 + nn])
                    else:
                        nc.scalar.copy(out_sb[:, t, n0:n0 + nn], fo_ps[:, 0:nn])
                half = NT // 2
                if t == half - 1 or t == NT - 1:
                    h0 = t + 1 - half
                    nc.gpsimd.dma_start(
                        out_d[b].rearrange("(t p) d -> p t d", p=128)[:, h0:t + 1, :],
                        out_sb[:, h0:t + 1, :])
                if t == NT - 1:
                    x_store.pop(b)

            # slot items: mid chunks then a/b halves with one-slot skew.
            # boundary runs in the NEXT batch's pre; late merges into its
            # first slot so p2b(7)'s softmax input has a pass1 tile of slack.
            slots = [
                [mid1], [mid2], [mid3],
                [mid4, lambda: p2a(0)],
                [lambda: p2a(1), lambda: p2b(0)],
                [lambda: p2a(2), lambda: p2b(1)],
                [lambda: p2a(3), lambda: p2b(2)],
                [lambda: p2a(4), lambda: p2b(3)],
            ]
            boundary = [lambda: p2a(5), lambda: p2b(4), lambda: p2a(6)]
            late = [lambda: p2b(5), lambda: p2a(7)]
            late2 = [lambda: p2b(6), lambda: p2b(7)]
            return [mid_cent], slots, boundary, late, late2

        # ---------------- pipelined emission ----------------
        # 1-tile software skew: softmax (DVE/ACT) of tile t overlaps the
        # fea/act GEMMs of tile t+1; cent of t lands after F(t+1) on the PE
        # queue. mid+pass2 of the previous batch fill the slots.
        def pass1_batch(b, pre, slots, post):
            for f in pre:
                f()
            si = 0
            for t in range(NT):
                seg_F(b, t)
                seg_S(b, t)
                if t > 0:
                    seg_C(b, t - 1)
                if si < len(slots):
                    for f in slots[si]:
                        f()
                    si += 1
            seg_C(b, NT - 1)
            for group in slots[si:]:
                for f in group:
                    f()
            for group in post:
                for f in group:
                    f()

        pass1_batch(0, [lambda: load_xt_batch(1)], [], [])
        carry_boundary, carry_late, carry_late2 = [], [], []
        for b in range(NB):
            pre, slots, boundary, late, late2 = make_items(b)
            if b + 1 < NB:
                pre2 = list(pre) + carry_boundary
                if b + 2 < NB:
                    pre2.append(lambda bb=b + 2: load_xt_batch(bb))
                slots2 = ([carry_late + slots[0], carry_late2 + slots[1]]
                          + slots[2:])
                pass1_batch(b + 1, pre2, slots2, [])
                carry_boundary, carry_late, carry_late2 = boundary, late, late2
            else:
                for f in carry_boundary + carry_late + carry_late2 + pre:
                    f()
                for group in slots:
                    for f in group:
                        f()
                for f in boundary + late + late2:
                    f()

    nc.compile()
    return nc


_PROGRAM_CACHE = {}


def _prep(inputs):
    """Host-side folds + layout packing. Returns (flags, common, xt_bf16)."""
    f32 = np.float32
    g = {k: np.ascontiguousarray(np.asarray(v, dtype=f32)) for k, v in inputs.items()}
    s1 = g["bn1_g"] / np.sqrt(g["bn1_v"] + f32(EPS))
    cwf = np.concatenate([g["cluster_weights"] * s1[None, :], g["W_ga"]], axis=1)
    bias1 = np.concatenate([g["bn1_b"] - g["bn1_m"] * s1, g["b_ga"]]) + g["b_exp"] @ cwf
    wcomb = g["W_exp"] @ cwf
    s2 = g["bn2_g"] / np.sqrt(g["bn2_v"] + f32(EPS))
    bias2 = (g["b_proj"][None, :] - g["bn2_m"][:, None]) * s2[:, None] + g["bn2_b"][:, None]
    flags = (
        bool(np.any(g["b_exp"])),
        bool(np.any(g["bq"])),
        bool(np.any(g["bkv"])),
        bool(np.any(g["bp2"])),
    )

    def ki_ko(w):
        """(ko*128+ki, n) fp32 -> bf16 [128 ki, ko, n] SBUF layout."""
        ko = w.shape[0] // 128
        return np.ascontiguousarray(
            w.reshape(ko, 128, w.shape[1]).transpose(1, 0, 2).astype(BF16NP))

    wproj_p = np.zeros((2, 128, D), f32)
    wproj_p[0] = g["W_proj"][0:128]
    wproj_p[1, 0:64] = g["W_proj"][128:GFS]
    common = {
        "wexp": ki_ko(g["W_exp"]),
        "wcomb": ki_ko(wcomb),
        "bias1": np.ascontiguousarray(bias1.astype(f32)),
        "wproj": np.ascontiguousarray(wproj_p.transpose(1, 0, 2).astype(BF16NP)),
        "s2": np.ascontiguousarray(s2.reshape(C, 1)),
        "bias2": np.ascontiguousarray(bias2),
        "wkv": ki_ko(g["Wkv"]),
        "wqT": ki_ko(np.ascontiguousarray(g["Wq"].T)),
        "wp2": ki_ko(g["Wp2"]),
    }
    if flags[0]:
        common["bexp"] = g["b_exp"]
    if flags[1]:
        common["bqT"] = ki_ko(g["bq"].reshape(P, 1))
    if flags[2]:
        common["bkv"] = g["bkv"]
    if flags[3]:
        common["bp2"] = g["bp2"]
    # x -> [b, ko, ki, s] bf16
    xt = np.ascontiguousarray(
        g["x"].reshape(B, S, KD, 128).transpose(0, 2, 3, 1).astype(BF16NP))
    return flags, common, xt


def run(inputs, trace=False):
    flags, common, xt = _prep(inputs)
    if flags not in _PROGRAM_CACHE:
        _PROGRAM_CACHE[flags] = build_program(flags)
    nc = _PROGRAM_CACHE[flags]
    in_maps = []
    for c in range(N_CORES):
        m = dict(common)
        m["xt"] = np.ascontiguousarray(xt[c * NB:(c + 1) * NB])
        in_maps.append(m)
    res = bass_utils.run_bass_kernel_spmd(
        nc, in_maps, core_ids=list(range(N_CORES)), trace=trace)
    out = np.concatenate([r["out"] for r in res.results], axis=0)
    return out, res


def kernel(**inputs):
    out, _ = run(inputs, trace=False)
    return out



# revision 7
# speedup vs baseline: 1.3345x; 1.3345x over previous
"""ClusterAttn Trainium2 kernel (Bass/Tile), 8-way data parallel over batch.

Full inputs in, full outputs out. Internally:
  - batch B=32 split 4-per-core across 8 NeuronCores (pure DP).
  - all PE compute in bf16 (fp32 psum accumulate); host casts inputs.
  - fea GEMM ELIMINATED: cent = einsum(act, fea_g) is re-associated as
    cent = sum_g (act_g^T @ x) @ W_exp_g. We compute T = x^T @ act
    (contract over tokens) then cent = sum_{g,k} T_slice^T @ W_exp_slice.
    act itself comes from the folded GEMM x @ (W_exp@[BN1-folded
    cluster_weights | W_ga]) as in the baseline.
  - attention pass2 computes scores TRANSPOSED (scT = wqk^T-slices @ x)
    so the out GEMM needs no attn transpose; softmax normalization is
    folded into the psum->sbuf copy via a ones-column appended to vw
    (row sums land in the psum, reciprocal gives 1/sum per token).
  - BN2 scale folded into the cent copy (per-cluster scale), BN2 bias
    folded host-side into bkv2 = bias2 @ Wkv + bkv.
  - x is host-packed in BOTH orientations: xt[b, ko, ki, s] (d on
    partitions: stationary/streaming for act+scores GEMMs) and
    xo[b, p, t, d] (tokens on partitions: stationary for the T GEMM).
"""

from contextlib import ExitStack

import numpy as np
import ml_dtypes

import concourse.bass as bass
import concourse.bacc as bacc
import concourse.tile as tile
import concourse.mybir as mybir
from concourse import bass_utils
from concourse.masks import make_identity

dt = mybir.dt
AF = mybir.ActivationFunctionType
ALU = mybir.AluOpType

EPS = 1e-5
N_CORES = 8
B, S, D = 32, 1024, 768
E, G, C, P = 2, 8, 64, 384
EF = E * D            # 1536
GC = G * C            # 512
GCG = GC + G          # 520
GFS = EF // G         # 192
NB = B // N_CORES     # batches per core
NT = S // 128         # token tiles per batch
KD = D // 128         # 6 contraction k-tiles over D
F32 = dt.float32
BF16 = dt.bfloat16
BF16NP = ml_dtypes.bfloat16


def build_program(flags):
    has_bexp, has_bq, has_bp2 = flags
    nc = bacc.Bacc(
        "TRN2",
        debug=False,
        enable_asserts=False,
        num_devices=N_CORES,
    )

    # x in both orientations (host-packed, bf16)
    xt_d = nc.dram_tensor("xt", (NB, KD, 128, S), BF16, kind="ExternalInput").ap()
    xo_d = nc.dram_tensor("xo", (NB, 128, NT, D), BF16, kind="ExternalInput").ap()
    out_d = nc.dram_tensor("out", (NB, S, D), F32, kind="ExternalOutput").ap()
    # weights already in SBUF layout [ki, ko, n]
    wexp_d = nc.dram_tensor("wexp", (128, KD, EF), BF16, kind="ExternalInput").ap()
    wcomb_d = nc.dram_tensor("wcomb", (128, KD, GCG), BF16, kind="ExternalInput").ap()
    bias1_d = nc.dram_tensor("bias1", (GCG,), F32, kind="ExternalInput").ap()
    wproj_d = nc.dram_tensor("wproj", (128, 2, D), BF16, kind="ExternalInput").ap()
    s2_d = nc.dram_tensor("s2", (C, 1), F32, kind="ExternalInput").ap()
    bkv2_d = nc.dram_tensor("bkv2", (C, 2 * P), F32, kind="ExternalInput").ap()
    wkv_d = nc.dram_tensor("wkv", (128, KD, 2 * P), BF16, kind="ExternalInput").ap()
    wqT_d = nc.dram_tensor("wqT", (128, 3, D), BF16, kind="ExternalInput").ap()
    wp2_d = nc.dram_tensor("wp2", (128, 3, D), BF16, kind="ExternalInput").ap()
    bexpg_d = bq_d = bp2_d = None
    if has_bexp:
        bexpg_d = nc.dram_tensor("bexpg", (G, GFS), BF16, kind="ExternalInput").ap()
    if has_bq:
        bq_d = nc.dram_tensor("bqT", (128, 3, 1), BF16, kind="ExternalInput").ap()
    if has_bp2:
        bp2_d = nc.dram_tensor("bp2", (2 * P,), F32, kind="ExternalInput").ap()

    inv_sqrt_p = float(1.0 / np.sqrt(np.float32(P)))

    with tile.TileContext(nc) as tc, ExitStack() as ctx:
        # ---------------- pools ----------------
        const = ctx.enter_context(tc.tile_pool(name="const", bufs=1))
        mid = ctx.enter_context(tc.tile_pool(name="mid", bufs=2))
        p_xt = ctx.enter_context(tc.tile_pool(name="p_xt", bufs=3))
        p_xo = ctx.enter_context(tc.tile_pool(name="p_xo", bufs=3))
        p_actf = ctx.enter_context(tc.tile_pool(name="p_actf", bufs=2))
        p_tsb = ctx.enter_context(tc.tile_pool(name="p_tsb", bufs=2))
        p_act = ctx.enter_context(tc.tile_pool(name="p_act", bufs=2))
        p_sm = ctx.enter_context(tc.tile_pool(name="p_sm", bufs=4))
        p_out = ctx.enter_context(tc.tile_pool(name="p_out", bufs=2))
        ps_T = ctx.enter_context(tc.tile_pool(name="ps_T", bufs=3, space="PSUM"))
        ps_act = ctx.enter_context(tc.tile_pool(name="ps_act", bufs=1, space="PSUM"))
        ps_mm = ctx.enter_context(tc.tile_pool(name="ps_mm", bufs=2, space="PSUM"))
        ps_t = ctx.enter_context(tc.tile_pool(name="ps_t", bufs=1, space="PSUM"))
        ps_misc = ctx.enter_context(tc.tile_pool(name="ps_misc", bufs=1, space="PSUM"))

        # ---------------- persistent PSUM regions ----------------
        misc_ps = ps_misc.tile([128, 512], F32, tag="misc")
        gate_ps = misc_ps[:, 0:8]          # gate logits accumulate per tile
        cent_ps = misc_ps[:, 64:256]       # [128,192] cent (2 col-halves)
        sct_ps = [misc_ps[0:64, 256:384], misc_ps[0:64, 384:512]]  # scoresT
        asumT_ps = misc_ps[64:128, 256:264]   # [64c, 8g] only if has_bexp
        bc_ps = misc_ps[64:128, 384:385]      # bias_c, only if has_bq

        # ---------------- constants ----------------
        x_store = {}   # b -> (xt, xo)

        def load_x_batch(b):
            xt = p_xt.tile([128, KD, S], BF16, tag="xt", name=f"xt{b}")
            nc.sync.dma_start(xt[:], xt_d[b].rearrange("ko ki s -> ki ko s"))
            xo = p_xo.tile([128, NT, D], BF16, tag="xo", name=f"xo{b}")
            nc.sync.dma_start(xo[:], xo_d[b])
            x_store[b] = (xt, xo)

        load_x_batch(0)

        ident = const.tile([128, 128], BF16)
        ident_f = const.tile([128, 128], F32)
        make_identity(nc, ident_f[:])
        nc.vector.tensor_copy(ident[:], ident_f[:])

        wexp_sb = const.tile([128, KD, EF], BF16)
        nc.sync.dma_start(wexp_sb[:], wexp_d)
        wcomb_sb = const.tile([128, KD, GCG], BF16)
        nc.sync.dma_start(wcomb_sb[:], wcomb_d)
        bias1_sb = const.tile([128, GCG], F32)
        nc.gpsimd.dma_start(bias1_sb[:], bias1_d.partition_broadcast(128))
        wproj_sb = const.tile([128, 2, D], BF16)
        nc.sync.dma_start(wproj_sb[:], wproj_d)
        s2_sb = const.tile([C, 1], F32)
        nc.sync.dma_start(s2_sb[:], s2_d)
        bkv2_sb = const.tile([C, 2 * P], F32)
        nc.sync.dma_start(bkv2_sb[:], bkv2_d)
        wkv_sb = const.tile([128, KD, 2 * P], BF16)
        nc.sync.dma_start(wkv_sb[:], wkv_d)
        wqT_sb = const.tile([128, 3, D], BF16)
        nc.sync.dma_start(wqT_sb[:], wqT_d)
        wp2_sb = const.tile([128, 3, D], BF16)
        nc.sync.dma_start(wp2_sb[:], wp2_d)
        if has_bexp:
            bexpg_sb = const.tile([G, GFS], BF16)
            nc.sync.dma_start(bexpg_sb[:], bexpg_d)
        if has_bq:
            bq_sb = const.tile([128, 3, 1], BF16)
            nc.sync.dma_start(bq_sb[:], bq_d)
        if has_bp2:
            bp2_sb = const.tile([C, 2 * P], F32)
            nc.gpsimd.dma_start(bp2_sb[:], bp2_d.partition_broadcast(C))

        def transpose_to(out_ps, in_ap, start=True, stop=True):
            kp = in_ap.partition_size()
            nc.tensor.matmul(
                out_ps,
                in_ap,
                ident[0:kp, 0:kp],
                is_transpose=True,
                start=start,
                stop=stop,
                skip_group_check=True,
            )

        # ---------------- per-batch state ----------------
        st = {}   # b -> dict

        def seg_act(b, t):
            """act/gate GEMM from xt; no fea."""
            xt, _ = x_store[b]
            ap = ps_act.tile([128, 512], F32, tag="amm")
            for k in range(KD):
                xk = xt[:, k, t * 128:(t + 1) * 128]
                nc.tensor.matmul(ap[:], xk, wcomb_sb[:, k, 0:512],
                                 start=(k == 0), stop=(k == KD - 1),
                                 skip_group_check=True)
                nc.tensor.matmul(gate_ps, xk, wcomb_sb[:, k, 512:520],
                                 start=(k == 0), stop=(k == KD - 1),
                                 skip_group_check=True)
            st[b]["ap"] = ap

        def seg_S(b, t):
            """grouped softmax * sigmoid gate -> actf[:, t, :] (bf16)."""
            sb = st[b]
            ap = sb.pop("ap")
            act = p_act.tile([128, GC], F32, tag="act")
            nc.vector.tensor_add(act[:], ap[:], bias1_sb[:, 0:GC])
            galog = p_sm.tile([128, G], F32, tag="galog")
            nc.vector.tensor_add(galog[:], gate_ps, bias1_sb[:, GC:GCG])
            e = p_act.tile([128, GC], F32, tag="e")
            nc.scalar.activation(e[:], act[:], AF.Exp)
            ssum = p_sm.tile([128, G], F32, tag="ssum")
            nc.vector.reduce_sum(ssum[:], e[:].rearrange("p (g c) -> p g c", g=G),
                                 axis=mybir.AxisListType.X)
            eneg = p_sm.tile([128, G], F32, tag="eneg")
            nc.scalar.activation(eneg[:], galog[:], AF.Exp, scale=-1.0)
            nc.vector.tensor_scalar_add(eneg[:], eneg[:], 1.0)
            ga = p_sm.tile([128, G], F32, tag="ga")
            nc.vector.reciprocal(ga[:], eneg[:])
            rs = p_sm.tile([128, G], F32, tag="rs")
            nc.vector.reciprocal(rs[:], ssum[:])
            nc.vector.tensor_mul(rs[:], rs[:], ga[:])
            actf = sb["actf"]
            nc.vector.tensor_tensor(
                out=actf[:, t, :].rearrange("p (g c) -> p g c", g=G),
                in0=e[:].rearrange("p (g c) -> p g c", g=G),
                in1=rs[:].unsqueeze(2).broadcast_to((128, G, C)),
                op=ALU.mult)

        def seg_T(b, t, dsl_list):
            """T GEMM: T[dsl] += xo[:,t,dsl]^T @ actf[:,t,:]."""
            sb = st[b]
            _, xo = x_store[b]
            actf = sb["actf"]
            for dsl in dsl_list:
                tp = sb["T"][dsl]
                nc.tensor.matmul(
                    tp[:], xo[:, t, dsl * 128:(dsl + 1) * 128], actf[:, t, :],
                    start=(t == 0), stop=(t == NT - 1),
                    skip_group_check=True)

        def alloc_T(b, dsls):
            for dsl in dsls:
                st[b]["T"][dsl] = ps_T.tile([128, 512], F32, tag="T",
                                            name=f"T{b}_{dsl}")

        def copy_T(b, dsls):
            sb = st[b]
            for dsl in dsls:
                tp = sb["T"].pop(dsl)
                nc.scalar.copy(sb["tsb"][:, dsl, :], tp[:])

        def sweepB(b, dsl):
            """second T chunk: one d-slice across all tiles."""
            sb = st[b]
            _, xo = x_store[b]
            actf = sb["actf"]
            tp = sb["T"][dsl]
            for t in range(NT):
                nc.tensor.matmul(
                    tp[:], xo[:, t, dsl * 128:(dsl + 1) * 128], actf[:, t, :],
                    start=(t == 0), stop=(t == NT - 1),
                    skip_group_check=True)

        def seg_asum(b, t):
            """only if has_bexp: asumT[c, g] += actf_g^T @ ones."""
            actf = st[b]["actf"]
            for g in range(G):
                nc.tensor.matmul(
                    asumT_ps[:, g:g + 1],
                    actf[:, t, g * 64:(g + 1) * 64],
                    ones_col[:],
                    start=(t == 0), stop=(t == NT - 1),
                    skip_group_check=True)

        if has_bexp:
            ones_col = const.tile([128, 1], BF16)
            nc.vector.memset(ones_col[:], 1.0)

        def seg_cent(b):
            """cent[c,f] = sum_{g,k} Tsb[:,k,g-slice]^T @ wexp[:,k,g-block].
            Even groups -> rows 0:64, odd -> rows 64:128 (col-tiled pairs);
            halves summed during the scaled copy."""
            sb = st[b]
            sb["tps"] = ps_t.tile([128, 512], BF16, tag="t", name=f"tps{b}")
            tsb = sb["tsb"]
            n_mm = 2 * KD * (G // 2)
            i = 0
            for gp in range(G // 2):
                for k in range(KD):
                    for half in range(2):
                        g = 2 * gp + half
                        out = cent_ps[64 * half:64 * half + 64, :]
                        stop = (i == n_mm - 1) or (
                            i == n_mm - 2 and not has_bexp)
                        nc.tensor.matmul(
                            out, tsb[:, k, g * 64:(g + 1) * 64],
                            wexp_sb[:, k, g * GFS:(g + 1) * GFS],
                            start=(i < 2), stop=stop,
                            skip_group_check=True)
                        i += 1
            if has_bexp:
                # transpose asumT [64,8] -> asum [8,64], add bias MM
                tps_b = sb["tps"]
                at_bf = p_sm.tile([C, G], BF16, tag="at_bf")
                nc.scalar.copy(at_bf[:], asumT_ps)
                transpose_to(tps_b[0:G, 448:512], at_bf[:])
                asum_sb = p_sm.tile([G, C], BF16, tag="asum_sb")
                nc.vector.tensor_copy(asum_sb[:], tps_b[0:G, 448:512])
                nc.tensor.matmul(cent_ps[0:C, :], asum_sb[:], bexpg_sb[:],
                                 start=False, stop=True, skip_group_check=True)
            # halves summed + BN2 scale folded in (cent * s2 per cluster row);
            # only one PSUM operand allowed per DVE op, so scale the even
            # half out via ACT first, then (odd * s2) + even_scaled on DVE.
            ctmp = p_sm.tile([C, GFS], F32, tag="ctmp")
            nc.scalar.activation(ctmp[:], cent_ps[0:64, :], AF.Copy,
                                 scale=s2_sb[:, 0:1])
            cents = mid.tile([C, GFS], BF16, tag="cents")
            nc.vector.scalar_tensor_tensor(
                out=cents[:], in0=cent_ps[64:128, :], scalar=s2_sb[:, 0:1],
                in1=ctmp[:], op0=ALU.mult, op1=ALU.add)
            sb["cents"] = cents

        def mid1(b):
            """centT transpose + nc2T = W_proj^T @ centT (BN2-scale already
            in cents; BN2 bias folded into bkv2)."""
            sb = st[b]
            cents = sb["cents"]
            tps_b = sb["tps"]
            transpose_to(tps_b[:, 0:64], cents[:, 0:128])
            transpose_to(tps_b[0:64, 64:128], cents[:, 128:192])
            centT = mid.tile([128, 2, C], BF16, tag="centT")
            nc.vector.tensor_copy(centT[:, 0, :], tps_b[:, 0:64])
            nc.vector.tensor_copy(centT[0:64, 1, :], tps_b[0:64, 64:128])
            n2p = ps_mm.tile([128, 512], F32, tag="mm")
            for dsl in range(KD):
                o = n2p[:, dsl * 64:(dsl + 1) * 64]
                nc.tensor.matmul(o, wproj_sb[:, 0, dsl * 128:(dsl + 1) * 128],
                                 centT[:, 0, :], start=True, stop=False,
                                 skip_group_check=True)
                nc.tensor.matmul(o, wproj_sb[0:64, 1, dsl * 128:(dsl + 1) * 128],
                                 centT[0:64, 1, :], start=False, stop=True,
                                 skip_group_check=True)
            nc2T = mid.tile([128, KD, C], BF16, tag="nc2T")
            nc.scalar.copy(nc2T[:].rearrange("p a b -> p (a b)"), n2p[:, 0:384])
            sb["nc2T"] = nc2T

        def mid2(b):
            """kv = nc2 @ Wkv + bkv2 (BN2 bias + bkv folded host-side)."""
            sb = st[b]
            nc2T = sb["nc2T"]
            kv = mid.tile([C, 2 * P], BF16, tag="kv")
            for n0, nn in ((0, 512), (512, 256)):
                kv_ps = ps_mm.tile([128, 512], F32, tag="mm")
                for k in range(KD):
                    nc.tensor.matmul(kv_ps[0:C, 0:nn], nc2T[:, k, :],
                                     wkv_sb[:, k, n0:n0 + nn],
                                     start=(k == 0), stop=(k == KD - 1))
                nc.vector.tensor_add(kv[:, n0:n0 + nn], kv_ps[0:C, 0:nn],
                                     bkv2_sb[:, n0:n0 + nn])
            sb["kv"] = kv

        def mid3(b):
            """kT, vT transposes."""
            sb = st[b]
            kv = sb["kv"]
            tps_b = sb["tps"]
            kT = mid.tile([128, 3, C], BF16, tag="kT")
            vT = mid.tile([128, 3, C], BF16, tag="vT")
            for i in range(3):
                transpose_to(tps_b[:, 128 + i * 64:128 + (i + 1) * 64],
                             kv[:, i * 128:(i + 1) * 128],
                             start=(i == 0), stop=(i == 2))
            nc.vector.tensor_copy(kT[:].rearrange("p a b -> p (a b)"),
                                  tps_b[:, 128:320])
            for i in range(3):
                transpose_to(tps_b[:, 320 + i * 64:320 + (i + 1) * 64],
                             kv[:, P + i * 128:P + (i + 1) * 128],
                             start=(i == 0), stop=(i == 2))
            nc.vector.tensor_copy(vT[:].rearrange("p a b -> p (a b)"),
                                  tps_b[:, 320:512])
            sb["kT"], sb["vT"] = kT, vT

        def mid4(b):
            """wqk = Wq @ k^T (rhs layout for scT GEMM) and vw = v @ Wp2
            with ones column appended (softmax-sum trick)."""
            sb = st[b]
            kT, vT = sb["kT"], sb["vT"]
            wq_ps = ps_mm.tile([128, 512], F32, tag="mm")
            for m in range(KD):
                for k3 in range(3):
                    nc.tensor.matmul(wq_ps[:, m * 64:(m + 1) * 64],
                                     wqT_sb[:, k3, m * 128:(m + 1) * 128],
                                     kT[:, k3, :], start=(k3 == 0),
                                     stop=(k3 == 2), skip_group_check=True)
            wqk = mid.tile([128, KD, C], BF16, tag="wqk")
            nc.scalar.copy(wqk[:].rearrange("p a b -> p (a b)"), wq_ps[:, 0:384])
            sb["wqk"] = wqk
            if has_bq:
                for k3 in range(3):
                    nc.tensor.matmul(bc_ps, bq_sb[:, k3, :], kT[:, k3, :],
                                     start=(k3 == 0), stop=(k3 == 2),
                                     skip_group_check=True)
                bias_c = p_sm.tile([C, 1], F32, tag="bias_c")
                nc.scalar.activation(bias_c[:], bc_ps, AF.Copy,
                                     scale=inv_sqrt_p)
                sb["bias_c"] = bias_c
            vw = mid.tile([C, 2 * P + 1], BF16, tag="vw")
            nc.vector.memset(vw[:, 2 * P:2 * P + 1], 1.0)
            for n0, nn in ((0, 512), (512, 256)):
                vw_ps = ps_mm.tile([128, 512], F32, tag="mm")
                for k3 in range(3):
                    nc.tensor.matmul(vw_ps[0:C, 0:nn], vT[:, k3, :],
                                     wp2_sb[:, k3, n0:n0 + nn],
                                     start=(k3 == 0), stop=(k3 == 2))
                if has_bp2:
                    nc.vector.tensor_add(vw[:, n0:n0 + nn], vw_ps[0:C, 0:nn],
                                         bp2_sb[:, n0:n0 + nn])
                else:
                    nc.scalar.copy(vw[:, n0:n0 + nn], vw_ps[0:C, 0:nn])
            sb["vw"] = vw
            sb["out_sb"] = None

        def p2a(b, tpair):
            """scoresT for two tiles: scT = wqk-slices^T stationary? no:
            lhsT = wqk[:,k,:] (64 cols), rhs = xt tile slice. Then exp."""
            sb = st[b]
            wqk = sb["wqk"]
            xt, _ = x_store[b]
            t0 = 2 * tpair
            for i, t in enumerate((t0, t0 + 1)):
                for k in range(KD):
                    nc.tensor.matmul(
                        sct_ps[i], wqk[:, k, :],
                        xt[:, k, t * 128:(t + 1) * 128],
                        start=(k == 0), stop=(k == KD - 1),
                        skip_group_check=True)
            for i, t in enumerate((t0, t0 + 1)):
                eT = p_sm.tile([C, 128], BF16, tag="eT", bufs=4,
                               name=f"eT{b}_{t}")
                if has_bq:
                    nc.scalar.activation(eT[:], sct_ps[i], AF.Exp,
                                         scale=inv_sqrt_p,
                                         bias=sb["bias_c"][:, 0:1])
                else:
                    nc.scalar.activation(eT[:], sct_ps[i], AF.Exp,
                                         scale=inv_sqrt_p)
                sb[("eT", t)] = eT

        def p2b(b, t):
            """out tile: fo = eT^T @ [vw | 1]; out = fo[:, :768] / fo[:, 768]."""
            sb = st[b]
            vw = sb["vw"]
            eT = sb.pop(("eT", t))
            if sb["out_sb"] is None:
                sb["out_sb"] = p_out.tile([128, NT // 2, D], F32, tag="out",
                                          name=f"out{b}_{t // 4}")
            out_sb = sb["out_sb"]
            fo = ps_mm.tile([128, 512], F32, tag="mm")
            fo2 = ps_mm.tile([128, 512], F32, tag="mm")
            nc.tensor.matmul(fo[:], eT[:], vw[:, 0:512], start=True, stop=True)
            nc.tensor.matmul(fo2[:, 0:257], eT[:], vw[:, 512:769],
                             start=True, stop=True)
            rs_a = p_sm.tile([128, 1], F32, tag="rs_a")
            nc.vector.reciprocal(rs_a[:], fo2[:, 256:257])
            tm = t % 4
            nc.scalar.activation(out_sb[:, tm, 0:512], fo[:],
                                 AF.Copy, scale=rs_a[:, 0:1])
            nc.scalar.activation(out_sb[:, tm, 512:768], fo2[:, 0:256],
                                 AF.Copy, scale=rs_a[:, 0:1])
            if tm == 3:
                h0 = t - 3
                nc.gpsimd.dma_start(
                    out_d[b].rearrange("(t p) d -> p t d", p=128)[:, h0:t + 1, :],
                    out_sb[:, :, :])
                sb["out_sb"] = None
            if t == NT - 1:
                x_store.pop(b)

        # ---------------- emission schedule ----------------
        # pass1(b) emits per tile: act GEMM + softmax, plus queued work:
        #  - slots 0-2: sweepB(b-1) d-slices 3..5  (T banks freed by copyA(b-1))
        #  - slot 2: copyB(b-1); slot 3: cent(b-1)
        #  - slots 4-7: T-chunkA MMs of THIS batch (tiles 0..6) + mid(b-1)
        #  - slot 7 + carry into pass1(b+1): p2a/p2b(b-1)
        def make_queue(b, first):
            """items queued into pass1(b)'s tile slots; work on batch b-1
            plus this batch's deferred T-A emissions."""
            q = [[] for _ in range(NT)]
            if first:
                # batch 0: T banks free from the start; straight 1-tile skew
                for t in range(NT - 1):
                    q[t + 1].append(lambda t=t: seg_T(b, t, (0, 1, 2)))
                q[1].insert(0, lambda: alloc_T(b, (0, 1, 2)))
                # prefetch next batch
                if b + 1 < NB:
                    q[0].append(lambda: load_x_batch(b + 1))
                return q, []
            bp = b - 1
            q[0].append(lambda: alloc_T(bp, (3,)))
            q[0].append(lambda: sweepB(bp, 3))
            q[1].append(lambda: alloc_T(bp, (4,)))
            q[1].append(lambda: sweepB(bp, 4))
            if b + 1 < NB:
                q[1].append(lambda: load_x_batch(b + 1))
            q[2].append(lambda: alloc_T(bp, (5,)))
            q[2].append(lambda: sweepB(bp, 5))
            q[2].append(lambda: copy_T(bp, (3, 4, 5)))
            q[3].append(lambda: seg_cent(bp))
            q[4].append(lambda: alloc_T(b, (0, 1, 2)))
            q[4].append(lambda: seg_T(b, 0, (0, 1, 2)))
            q[4].append(lambda: seg_T(b, 1, (0, 1, 2)))
            q[4].append(lambda: mid1(bp))
            q[5].append(lambda: seg_T(b, 2, (0, 1, 2)))
            q[5].append(lambda: seg_T(b, 3, (0, 1, 2)))
            q[5].append(lambda: mid2(bp))
            q[5].append(lambda: mid3(bp))
            q[6].append(lambda: seg_T(b, 4, (0, 1, 2)))
            q[6].append(lambda: seg_T(b, 5, (0, 1, 2)))
            q[6].append(lambda: mid4(bp))
            q[7].append(lambda: seg_T(b, 6, (0, 1, 2)))
            q[7].append(lambda: p2a(bp, 0))
            carry = [
                [lambda: p2b(bp, 0), lambda: p2a(bp, 1)],
                [lambda: p2b(bp, 1), lambda: p2a(bp, 2)],
                [lambda: p2b(bp, 2), lambda: p2b(bp, 3)],
                [lambda: p2a(bp, 3)],
                [lambda: p2b(bp, 4), lambda: p2b(bp, 5)],
                [lambda: p2b(bp, 6), lambda: p2b(bp, 7)],
            ]
            return q, carry

        def pass1(b, queue, carry_in):
            st[b] = {"T": {}, "actf": p_actf.tile([128, NT, GC], BF16,
                                                  tag="actf", name=f"actf{b}"),
                     "tsb": p_tsb.tile([128, KD, GC], BF16, tag="tsb",
                                       name=f"tsb{b}")}
            for t in range(NT):
                seg_act(b, t)
                if has_bexp and t > 0:
                    seg_asum(b, t - 1)
                seg_S(b, t)
                for f in queue[t]:
                    f()
                if t < len(carry_in):
                    for f in carry_in[t]:
                        f()
            if has_bexp:
                seg_asum(b, NT - 1)
            # T chunk A tail + copy
            seg_T(b, NT - 1, (0, 1, 2))
            copy_T(b, (0, 1, 2))

        carry = []
        for b in range(NB):
            queue, new_carry = make_queue(b, first=(b == 0))
            pass1(b, queue, carry)
            carry = new_carry
        # drain the last batch's mid+pass2 serially
        bl = NB - 1
        for grp in carry:
            for f in grp:
                f()
        alloc_T(bl, (3,)); sweepB(bl, 3)
        alloc_T(bl, (4,)); sweepB(bl, 4)
        alloc_T(bl, (5,)); sweepB(bl, 5)
        copy_T(bl, (3, 4, 5))
        seg_cent(bl)
        mid1(bl)
        mid2(bl)
        mid3(bl)
        mid4(bl)
        p2a(bl, 0)
        p2a(bl, 1)
        p2b(bl, 0)
        p2b(bl, 1)
        p2a(bl, 2)
        p2b(bl, 2)
        p2b(bl, 3)
        p2a(bl, 3)
        p2b(bl, 4)
        p2b(bl, 5)
        p2b(bl, 6)
        p2b(bl, 7)

    nc.compile()
    return nc


_PROGRAM_CACHE = {}


def _prep(inputs):
    """Host-side folds + layout packing. Returns (flags, common, per_core)."""
    f32 = np.float32
    g = {k: np.ascontiguousarray(np.asarray(v, dtype=f32)) for k, v in inputs.items()}
    s1 = g["bn1_g"] / np.sqrt(g["bn1_v"] + f32(EPS))
    cwf = np.concatenate([g["cluster_weights"] * s1[None, :], g["W_ga"]], axis=1)
    bias1 = np.concatenate([g["bn1_b"] - g["bn1_m"] * s1, g["b_ga"]]) + g["b_exp"] @ cwf
    wcomb = g["W_exp"] @ cwf
    s2 = g["bn2_g"] / np.sqrt(g["bn2_v"] + f32(EPS))
    bias2 = (g["b_proj"][None, :] - g["bn2_m"][:, None]) * s2[:, None] + g["bn2_b"][:, None]
    bkv2 = bias2 @ g["Wkv"] + g["bkv"][None, :]
    flags = (
        bool(np.any(g["b_exp"])),
        bool(np.any(g["bq"])),
        bool(np.any(g["bp2"])),
    )

    def ki_ko(w):
        ko = w.shape[0] // 128
        return np.ascontiguousarray(
            w.reshape(ko, 128, w.shape[1]).transpose(1, 0, 2).astype(BF16NP))

    wproj_p = np.zeros((2, 128, D), f32)
    wproj_p[0] = g["W_proj"][0:128]
    wproj_p[1, 0:64] = g["W_proj"][128:GFS]
    common = {
        "wexp": ki_ko(g["W_exp"]),
        "wcomb": ki_ko(wcomb),
        "bias1": np.ascontiguousarray(bias1.astype(f32)),
        "wproj": np.ascontiguousarray(wproj_p.transpose(1, 0, 2).astype(BF16NP)),
        "s2": np.ascontiguousarray(s2.reshape(C, 1)),
        "bkv2": np.ascontiguousarray(bkv2.astype(f32)),
        "wkv": ki_ko(g["Wkv"]),
        "wqT": ki_ko(np.ascontiguousarray(g["Wq"].T)),
        "wp2": ki_ko(g["Wp2"]),
    }
    if flags[0]:
        common["bexpg"] = np.ascontiguousarray(
            g["b_exp"].reshape(G, GFS).astype(BF16NP))
    if flags[1]:
        common["bqT"] = ki_ko(g["bq"].reshape(P, 1))
    if flags[2]:
        common["bp2"] = np.ascontiguousarray(
            np.broadcast_to(g["bp2"], (2 * P,)).astype(f32))
    # x -> both layouts, bf16
    xt = np.ascontiguousarray(
        g["x"].reshape(B, S, KD, 128).transpose(0, 2, 3, 1).astype(BF16NP))
    xo = np.ascontiguousarray(
        g["x"].reshape(B, NT, 128, D).transpose(0, 2, 1, 3).astype(BF16NP))
    per_core = []
    for c in range(N_CORES):
        per_core.append({
            "xt": np.ascontiguousarray(xt[c * NB:(c + 1) * NB]),
            "xo": np.ascontiguousarray(xo[c * NB:(c + 1) * NB]),
        })
    return flags, common, per_core


def make_in_maps(flags, common, per_core):
    in_maps = []
    for c in range(N_CORES):
        m = dict(common)
        m.update(per_core[c])
        in_maps.append(m)
    return in_maps


def run(inputs, trace=False):
    flags, common, per_core = _prep(inputs)
    if flags not in _PROGRAM_CACHE:
        _PROGRAM_CACHE[flags] = build_program(flags)
    nc = _PROGRAM_CACHE[flags]
    in_maps = make_in_maps(flags, common, per_core)
    res = bass_utils.run_bass_kernel_spmd(
        nc, in_maps, core_ids=list(range(N_CORES)), trace=trace)
    out = np.concatenate([r["out"] for r in res.results], axis=0)
    return out, res


def kernel(**inputs):
    out, _ = run(inputs, trace=False)
    return out
